# revision 1
# baseline (speedup 1.0000x reference)
"""Trainium2 Bass kernel for nn_AttnBlock (GroupNorm + single-head attention + proj + residual).

Reference computation (per batch element b, with C=256 channels, N=64*64=4096 positions):
    h   = GroupNorm32(x) * gn_scale + gn_bias
    q,k,v = split(qkv_w @ h + qkv_b)          (channel-interleaved split: rows 3c+0/1/2)
    w   = softmax_k(q^T k / sqrt(C))          [N, N]
    a   = v @ w^T                             [C, N]
    out = proj_w @ a + proj_b + x

Sharding: 8 cores = 4 batches x 2 q-halves.  Each core gets one full batch
element (needed for GroupNorm stats and full k/v), rolled so that its own
q-half occupies columns 0:2048; it computes the attention output for those
2048 query positions only.

Device algorithm (per core):
  - GroupNorm stats via bn_stats/bn_aggr + tiny indicator matmuls (group
    reduce + broadcast across partitions).
  - GN is folded into the qkv weights on-chip: W' = W.T * scale_c (per-input-
    channel), bias chain beta_W = W.T @ bias_c computed with tiny matmuls, so
    `h` is never materialized; all projections read raw x.
  - Scores are computed transposed (k-position on partitions): sT[kt, q] so
    that exp(sT) tiles feed the a=v@w matmul directly without transposes.
  - Softmax normalization is deferred: av and rowsum accumulate over 32
    k-tiles in PSUM (rowsum via a ones[128,128] stationary matmul), then
    a = av * (1/rowsum).
  - v-bias (GN part) is folded into the output bias via delta = P^T @ (Wv^T @
    bias_c); the constant part proj_b + proj_w @ bv is folded on the host.
  - All big matmuls run as float32r (full-rate fp32 mode, free dim >= 256).
    fp32r operands must be produced rounded: operand tiles are allocated with
    dtype float32r and written by a compute engine (ACT/DVE), which applies
    the rounding; raw x gets an explicit DVE rounding copy.
"""

import numpy as np

import concourse.bass as bass
import concourse.bacc as bacc
import concourse.tile as tile
from concourse import mybir
from concourse.bass_utils import run_bass_kernel_spmd

F32 = mybir.dt.float32
F32R = mybir.dt.float32r
AF = mybir.ActivationFunctionType
OP = mybir.AluOpType

B, C, H, W = 4, 256, 64, 64
N = H * W               # 4096 positions
NQ = N // 2             # 2048 query positions per core
GROUPS = 32
GSIZE = C // GROUPS     # 8 channels per group
EPS = 1e-6
QB = 512                # query block (one PSUM bank of fp32)
NJB = NQ // QB          # 4 query blocks
KT = N // 128           # 32 k-position tiles
NCORES = 8


def _indicator_constants():
    # gind: [128, 2, 32] (partition-major) with gind[p, t, g] = 1 iff
    #   group(t*128+p) == g;  gindT[t]: [32, 128] transpose (for broadcasting
    #   group stats back to channels)
    p = np.arange(128)
    gind = np.zeros((2, 128, 32), np.float32)
    for t in range(2):
        gind[t, p, t * 16 + p // GSIZE] = 1.0
    gindT = np.ascontiguousarray(np.transpose(gind, (0, 2, 1)))
    # gind pre-scaled by 1/GSIZE so the group-reduce matmul yields means
    gind_pmaj = np.ascontiguousarray(
        np.transpose(gind, (1, 0, 2))).reshape(128, 64) / GSIZE
    return gind_pmaj.astype(np.float32), gindT.reshape(2 * 32, 128)


def _emit(nc, tc, d):
    """Emit the per-core program. d: dict of DRAM APs."""
    x_d, wq_d, wk_d, wv_d, pt_d = d["x"], d["wqT"], d["wkT"], d["wvT"], d["pT"]
    vec_d, out_d = d["vecs"], d["out"]
    gind_d, gindT_d = d["gind"], d["gindT"]

    import contextlib
    ctx = contextlib.ExitStack()
    with ctx:
        sing = ctx.enter_context(tc.tile_pool(name="sing", bufs=1))
        stat = ctx.enter_context(tc.tile_pool(name="stat", bufs=2))

        # ---- persistent SBUF tiles -------------------------------------
        x0 = sing.tile([128, N], F32, name="x0")
        x1 = sing.tile([128, N], F32, name="x1")
        xr0 = sing.tile([128, N], F32R, name="xr0")   # fp32r-rounded copy of x
        xr1 = sing.tile([128, N], F32R, name="xr1")
        k0 = sing.tile([128, N], F32R, name="k0")
        k1 = sing.tile([128, N], F32R, name="k1")
        q0 = sing.tile([128, NQ], F32R, name="q0")
        q1 = sing.tile([128, NQ], F32R, name="q1")
        vt = sing.tile([128, KT, 256], F32R, name="vt")
        wq = sing.tile([128, 2, 256], F32, name="wq")   # [c_in_part, chunk, c_out]
        wk = sing.tile([128, 2, 256], F32, name="wk")
        wv = sing.tile([128, 2, 256], F32, name="wv")
        pt = sing.tile([128, 2, 256], F32, name="pt")
        wqs = sing.tile([128, 2, 256], F32R, name="wqs")  # GN-scaled, fp32r
        wks = sing.tile([128, 2, 256], F32R, name="wks")
        wvs = sing.tile([128, 2, 256], F32R, name="wvs")
        ptr = sing.tile([128, 2, 256], F32R, name="ptr")
        vecs = sing.tile([128, 5, 2], F32, name="vecs")  # gn_scale, gn_bias, bq, bk, pbe
        gind = sing.tile([128, 2, 32], F32, name="gind")
        gindT0 = sing.tile([32, 128], F32, name="gindT0")
        gindT1 = sing.tile([32, 128], F32, name="gindT1")
        ones_f = sing.tile([128, 128], F32, name="ones_f")
        ones1 = sing.tile([128, 1], F32R, name="ones1")
        epst = sing.tile([32, 1], F32, name="epst")

        scale_c = sing.tile([128, 2], F32, name="scale_c")   # per-channel GN scale
        gnb_c = sing.tile([128, 2], F32, name="gnb_c")       # per-channel GN bias
        bq_t = sing.tile([128, 2], F32, name="bq_t")         # q bias per c_out
        bk_t = sing.tile([128, 2], F32, name="bk_t")
        bv_t = sing.tile([128, 2], F32, name="bv_t")
        ob_t = sing.tile([128, 2], F32, name="ob_t")         # final output bias

        # ---- DMAs -------------------------------------------------------
        # x chunks split over two DGE queues (sync/gpsimd) so the two tiles
        # stream in parallel; small tensors go via the tensor engine's queue.
        # bn_stats / fp32r casts start while later chunks are in flight.
        XCH = 1024
        for c in range(N // XCH):
            csl = slice(c * XCH, (c + 1) * XCH)
            nc.sync.dma_start(out=x0[:, csl], in_=x_d[0:128, csl])
            nc.scalar.dma_start(out=x1[:, csl], in_=x_d[128:256, csl])
        for wt, wd in ((wq, wq_d), (wk, wk_d)):
            nc.sync.dma_start(out=wt, in_=wd.rearrange("(j p) o -> p j o", p=128))
        for wt, wd in ((wv, wv_d), (pt, pt_d)):
            nc.scalar.dma_start(out=wt, in_=wd.rearrange("(j p) o -> p j o", p=128))
        nc.gpsimd.dma_start(out=vecs, in_=vec_d)
        nc.gpsimd.dma_start(out=gind, in_=gind_d)
        nc.gpsimd.dma_start(out=gindT0, in_=gindT_d[0:32, :])
        nc.gpsimd.dma_start(out=gindT1, in_=gindT_d[32:64, :])
        nc.vector.memset(ones_f, 1.0)
        nc.vector.tensor_copy(out=ones1, in_=ones_f[:, 0:1])
        nc.vector.memset(epst, EPS)

        # fp32r rounding copies: x on ACT (chunked, overlaps DMA), rest tiny
        for c in range(N // XCH):
            csl = slice(c * XCH, (c + 1) * XCH)
            nc.scalar.copy(out=xr0[:, csl], in_=x0[:, csl])
            nc.scalar.copy(out=xr1[:, csl], in_=x1[:, csl])

        gsc = vecs[:, 0, :]
        gbi = vecs[:, 1, :]
        bqv = vecs[:, 2, :]
        bkv = vecs[:, 3, :]
        pbe = vecs[:, 4, :]

        # ---- phase 1: GroupNorm statistics ------------------------------
        with tc.tile_pool(name="ps_small", bufs=2, space="PSUM") as ps_small:
            # bn_stats interleaved x0/x1 in DMA-chunk arrival order
            bstats0 = stat.tile([128, GSIZE, 6], F32, name="bstats0", tag="bstats0", bufs=1)
            bstats1 = stat.tile([128, GSIZE, 6], F32, name="bstats1", tag="bstats1", bufs=1)
            for sg in range(GSIZE):
                nc.vector.bn_stats(out=bstats0[:, sg, :], in_=x0[:, sg * 512:(sg + 1) * 512])
                nc.vector.bn_stats(out=bstats1[:, sg, :], in_=x1[:, sg * 512:(sg + 1) * 512])
            statsin = []
            for t, bstats in enumerate((bstats0, bstats1)):
                mv = stat.tile([128, 2], F32, name=f"mv{t}", tag="mv")
                nc.vector.bn_aggr(out=mv, in_=bstats)
                # statsin = (mean_c, E[x^2]_c)
                si = stat.tile([128, 2], F32, name=f"si{t}", tag=f"si{t}", bufs=1)
                nc.vector.tensor_copy(out=si[:, 0:1], in_=mv[:, 0:1])
                nc.vector.tensor_tensor(out=si[:, 1:2], in0=mv[:, 0:1], in1=mv[:, 0:1], op=OP.mult)
                nc.vector.tensor_tensor(out=si[:, 1:2], in0=si[:, 1:2], in1=mv[:, 1:2], op=OP.add)
                statsin.append(si)

            gsum_ps = ps_small.tile([32, 2], F32, name="gsum_ps", tag="gsum")
            nc.tensor.matmul(gsum_ps, gind[:, 0, :], statsin[0], start=True, stop=False)
            nc.tensor.matmul(gsum_ps, gind[:, 1, :], statsin[1], start=False, stop=True)

            # group mean / E[x^2] -> (mu_g, rstd_g); gind is pre-scaled by
            # 1/GSIZE so gsum_ps already holds (mu_g, E2_g)
            grp = stat.tile([32, 2], F32, name="grp", bufs=1)
            nc.vector.tensor_copy(out=grp, in_=gsum_ps)
            var_g = stat.tile([32, 1], F32, name="var_g", bufs=1)
            # mu^2 - E2 = -var, then sqrt(-1 * in + eps) = sqrt(var + eps)
            nc.vector.scalar_tensor_tensor(out=var_g, in0=grp[:, 0:1],
                                           scalar=grp[:, 0:1], in1=grp[:, 1:2],
                                           op0=OP.mult, op1=OP.subtract)
            nc.scalar.activation(out=var_g, in_=var_g, func=AF.Sqrt, bias=epst, scale=-1.0)
            nc.vector.reciprocal(out=grp[:, 1:2], in_=var_g)  # grp = (mu_g, rstd_g)

            for t, gt in enumerate((gindT0, gindT1)):
                bc_ps = ps_small.tile([128, 2], F32, name=f"bc_ps{t}", tag="bc")
                nc.tensor.matmul(bc_ps, gt, grp, start=True, stop=True)
                # scale_c = gn_scale * rstd ; gnb_c = gn_bias - mu * scale_c
                # (DVE reads bc_ps straight from PSUM — no staging copy)
                nc.vector.tensor_tensor(out=scale_c[:, t:t + 1], in0=gsc[:, t:t + 1],
                                        in1=bc_ps[:, 1:2], op=OP.mult)
                nc.vector.tensor_tensor(out=gnb_c[:, t:t + 1], in0=bc_ps[:, 0:1],
                                        in1=scale_c[:, t:t + 1], op=OP.mult)
                nc.vector.tensor_tensor(out=gnb_c[:, t:t + 1], in0=gbi[:, t:t + 1],
                                        in1=gnb_c[:, t:t + 1], op=OP.subtract)

            # ---- phase 2: fold GN scale into qkv weights (fp32r on write)
            for wsrc, wdst in ((wq, wqs), (wk, wks), (wv, wvs)):
                for cchunk in range(2):
                    nc.vector.tensor_scalar_mul(out=wdst[:, cchunk, :], in0=wsrc[:, cchunk, :],
                                                scalar1=scale_c[:, cchunk:cchunk + 1])

            # bias chains: beta_W = W^T @ gnb (+ input bias)
            for wt, bsrc, bdst in ((wk, bkv, bk_t), (wq, bqv, bq_t), (wv, None, bv_t)):
                for ot in range(2):
                    b_ps = ps_small.tile([128, 1], F32, name=f"b_ps{ot}", tag="b_ps")
                    nc.tensor.matmul(b_ps, wt[:, 0, ot * 128:(ot + 1) * 128],
                                     gnb_c[:, 0:1], start=True, stop=False)
                    nc.tensor.matmul(b_ps, wt[:, 1, ot * 128:(ot + 1) * 128],
                                     gnb_c[:, 1:2], start=False, stop=True)
                    if bsrc is not None:
                        nc.vector.tensor_tensor(out=bdst[:, ot:ot + 1], in0=b_ps,
                                                in1=bsrc[:, ot:ot + 1], op=OP.add)
                    else:
                        nc.vector.tensor_copy(out=bdst[:, ot:ot + 1], in_=b_ps)
            # delta = P^T @ beta_v ; out bias = pbe + delta
            for ot in range(2):
                d_ps = ps_small.tile([128, 1], F32, name=f"d_ps{ot}", tag="b_ps")
                nc.tensor.matmul(d_ps, pt[:, 0, ot * 128:(ot + 1) * 128],
                                 bv_t[:, 0:1], start=True, stop=False)
                nc.tensor.matmul(d_ps, pt[:, 1, ot * 128:(ot + 1) * 128],
                                 bv_t[:, 1:2], start=False, stop=True)
                nc.vector.tensor_tensor(out=ob_t[:, ot:ot + 1], in0=d_ps,
                                        in1=pbe[:, ot:ot + 1], op=OP.add)

        # ---- phase 3: q / k / vT projections ----------------------------
        # interleave the vT matmuls (N=256, LDWEIGHTS-bound: x-slice loads)
        # with the q/k matmuls (N=512, MM-bound) so the weight loads hide
        # under the streaming of the big matmuls
        nc.gpsimd.tensor_copy(out=ptr, in_=pt)
        with tc.tile_pool(name="ps_proj3", bufs=4, space="PSUM") as ps3:
            big = []  # (dst, bias, weight, chunk, jb, engine) — k first
            for ot, kt_sb in enumerate((k0, k1)):
                for jb in range(N // QB):
                    big.append((kt_sb, bk_t[:, ot:ot + 1], wks, ot, jb, "act"))
            for ot, qt in enumerate((q0, q1)):
                for jb in range(NJB):
                    big.append((qt, bq_t[:, ot:ot + 1], wqs, ot, jb, "act"))

            for nt in range(KT):
                if big and nt % 4 != 3:   # 24 big blocks over 32 nt slots
                    dst, bias, wgt, ot, jb, eng = big.pop(0)
                    sl = slice(jb * QB, (jb + 1) * QB)
                    p_b = ps3.tile([128, QB], F32, name="p_b", tag="pp")
                    nc.tensor.matmul(p_b, wgt[:, 0, ot * 128:(ot + 1) * 128],
                                     xr0[:, sl], start=True, stop=False)
                    nc.tensor.matmul(p_b, wgt[:, 1, ot * 128:(ot + 1) * 128],
                                     xr1[:, sl], start=False, stop=True)
                    if eng == "act":
                        nc.scalar.activation(out=dst[:, sl], in_=p_b, func=AF.Identity,
                                             bias=bias, scale=1.0)
                    else:
                        nc.vector.tensor_scalar_add(out=dst[:, sl], in0=p_b, scalar1=bias)
                nsl = slice(nt * 128, (nt + 1) * 128)
                p_v = ps3.tile([128, 256], F32, name="p_v", tag="pp")
                nc.tensor.matmul(p_v, xr0[:, nsl], wvs[:, 0, :], start=True, stop=False)
                nc.tensor.matmul(p_v, xr1[:, nsl], wvs[:, 1, :], start=False, stop=True)
                nc.vector.tensor_copy(out=vt[:, nt, :], in_=p_v)
            assert not big

        # ---- phase 4: attention -----------------------------------------
        with (
            tc.tile_pool(name="ps_s", bufs=3, space="PSUM") as ps_s,
            tc.tile_pool(name="ps_av", bufs=3, space="PSUM") as ps_av,
            tc.tile_pool(name="ps_po", bufs=2, space="PSUM") as ps_po,
            tc.tile_pool(name="eT_pool", bufs=8) as eT_pool,
            tc.tile_pool(name="an_pool", bufs=4) as an_pool,
            tc.tile_pool(name="o_pool", bufs=4) as o_pool,
            tc.tile_pool(name="rs_pool", bufs=2) as rs_pool,
        ):
            def epilogue(jb, an_a, an_b):
                # proj matmuls + bias/residual + store for query block jb.
                # Deferred into the NEXT block's kt loop so the PE stream
                # never stalls waiting on the DVE normalize chain.
                qsl = slice(jb * QB, (jb + 1) * QB)
                for ot, xres in enumerate((x0, x1)):
                    po = ps_po.tile([128, QB], F32, name="po", tag="po")
                    nc.tensor.matmul(po, ptr[:, 0, ot * 128:(ot + 1) * 128],
                                     an_a, start=True, stop=False)
                    nc.tensor.matmul(po, ptr[:, 1, ot * 128:(ot + 1) * 128],
                                     an_b, start=False, stop=True)
                    o_sb = o_pool.tile([128, QB], F32, name="o_sb", tag="o_sb")
                    # out = (po + out_bias) + x_residual
                    nc.vector.scalar_tensor_tensor(out=o_sb, in0=po,
                                                   scalar=ob_t[:, ot:ot + 1],
                                                   in1=xres[:, qsl],
                                                   op0=OP.add, op1=OP.add)
                    nc.sync.dma_start(out=out_d[ot * 128:(ot + 1) * 128, qsl], in_=o_sb)

            pending = None
            for jb in range(NJB):
                qsl = slice(jb * QB, (jb + 1) * QB)
                av_a = ps_av.tile([128, QB], F32, name="av_a", tag="av")
                av_b = ps_av.tile([128, QB], F32, name="av_b", tag="av")
                rs = ps_av.tile([128, QB], F32, name="rs", tag="av")
                # 2-deep software pipeline: scores(kt) issue ahead of av(kt-2)
                # so the PE never waits on the exp latency
                eTs = {}

                def av_group(kt):
                    eT = eTs.pop(kt)
                    # rowsum first (M=1, near-zero LDWEIGHTS) so the final
                    # reciprocal chain can start before the last av matmuls
                    nc.tensor.matmul(rs[0:1, :], ones1, eT,
                                     start=(kt == 0), stop=(kt == KT - 1))
                    nc.tensor.matmul(av_a, vt[:, kt, 0:128], eT,
                                     start=(kt == 0), stop=(kt == KT - 1))
                    nc.tensor.matmul(av_b, vt[:, kt, 128:256], eT,
                                     start=(kt == 0), stop=(kt == KT - 1))

                for kt in range(KT):
                    ksl = slice(kt * 128, (kt + 1) * 128)
                    s_ps = ps_s.tile([128, QB], F32, name="s_ps", tag="s")
                    nc.tensor.matmul(s_ps, k0[:, ksl], q0[:, qsl], start=True, stop=False)
                    nc.tensor.matmul(s_ps, k1[:, ksl], q1[:, qsl], start=False, stop=True)
                    eT = eT_pool.tile([128, QB], F32R, name="eT", tag="eT")
                    nc.scalar.activation(out=eT, in_=s_ps, func=AF.Exp)
                    eTs[kt] = eT
                    if kt >= 2:
                        av_group(kt - 2)
                    if kt == 4 and pending is not None:
                        epilogue(*pending)
                        pending = None
                av_group(KT - 2)
                av_group(KT - 1)
                if jb < NJB - 1:
                    # normalize: a = av * (1/rowsum) — DVE reciprocal of the
                    # [1,Q] rowsum, gpsimd broadcast across partitions;
                    # overlaps the next block's PE work
                    rsr = rs_pool.tile([1, QB], F32, name="rsr", tag="rsr")
                    nc.vector.reciprocal_approx_fast(out=rsr, in_=rs[0:1, :])
                    rsb = rs_pool.tile([128, QB], F32, name="rsb", tag="rsb")
                    nc.gpsimd.partition_broadcast(rsb, rsr)
                    an_a = an_pool.tile([128, QB], F32R, name="an_a", tag="an")
                    an_b = an_pool.tile([128, QB], F32R, name="an_b", tag="an")
                    nc.vector.tensor_tensor(out=an_a, in0=av_a, in1=rsb, op=OP.mult)
                    nc.vector.tensor_tensor(out=an_b, in0=av_b, in1=rsb, op=OP.mult)
                    pending = (jb, an_a, an_b)
                else:
                    # final block: no following PE work to hide behind, so
                    # pipeline the normalize/proj/store chain in two
                    # half-width pieces (DVE of half 2 overlaps PE of half 1)
                    HB = QB // 2
                    for h in range(2):
                        hsl = slice(h * HB, (h + 1) * HB)
                        qsl_h = slice(jb * QB + h * HB, jb * QB + (h + 1) * HB)
                        rsr_h = rs_pool.tile([1, HB], F32, name=f"rsrh{h}", tag=f"rsrh{h}", bufs=1)
                        nc.vector.reciprocal_approx_fast(out=rsr_h, in_=rs[0:1, hsl])
                        rsb_h = rs_pool.tile([128, HB], F32, name=f"rsbh{h}", tag=f"rsbh{h}", bufs=1)
                        nc.gpsimd.partition_broadcast(rsb_h, rsr_h)
                        an_ah = an_pool.tile([128, HB], F32R, name=f"an_ah{h}", tag="an")
                        an_bh = an_pool.tile([128, HB], F32R, name=f"an_bh{h}", tag="an")
                        nc.vector.tensor_tensor(out=an_ah, in0=av_a[:, hsl], in1=rsb_h, op=OP.mult)
                        nc.vector.tensor_tensor(out=an_bh, in0=av_b[:, hsl], in1=rsb_h, op=OP.mult)
                        for ot, xres in enumerate((x0, x1)):
                            po = ps_po.tile([128, HB], F32, name="po_h", tag="po")
                            nc.tensor.matmul(po, ptr[:, 0, ot * 128:(ot + 1) * 128],
                                             an_ah, start=True, stop=False)
                            nc.tensor.matmul(po, ptr[:, 1, ot * 128:(ot + 1) * 128],
                                             an_bh, start=False, stop=True)
                            o_sb = o_pool.tile([128, HB], F32, name="o_sb_h", tag="o_sb")
                            nc.vector.scalar_tensor_tensor(out=o_sb, in0=po,
                                                           scalar=ob_t[:, ot:ot + 1],
                                                           in1=xres[:, qsl_h],
                                                           op0=OP.add, op1=OP.add)
                            nc.sync.dma_start(out=out_d[ot * 128:(ot + 1) * 128, qsl_h],
                                              in_=o_sb)
            assert pending is None


_CACHED_NC = None


def _build_program():
    global _CACHED_NC
    if _CACHED_NC is not None:
        return _CACHED_NC
    nc = bacc.Bacc("TRN2", target_bir_lowering=False, debug=False,
                   num_devices=NCORES)
    d = {
        "x": nc.dram_tensor("x", [C, N], F32, kind="ExternalInput").ap(),
        "wqT": nc.dram_tensor("wqT", [C, C], F32, kind="ExternalInput").ap(),
        "wkT": nc.dram_tensor("wkT", [C, C], F32, kind="ExternalInput").ap(),
        "wvT": nc.dram_tensor("wvT", [C, C], F32, kind="ExternalInput").ap(),
        "pT": nc.dram_tensor("pT", [C, C], F32, kind="ExternalInput").ap(),
        "vecs": nc.dram_tensor("vecs", [128, 10], F32, kind="ExternalInput").ap(),
        "gind": nc.dram_tensor("gind", [128, 64], F32, kind="ExternalInput").ap(),
        "gindT": nc.dram_tensor("gindT", [2 * 32, 128], F32, kind="ExternalInput").ap(),
        "out": nc.dram_tensor("out", [C, NQ], F32, kind="ExternalOutput").ap(),
    }
    with tile.TileContext(nc) as tc:
        _emit(nc, tc, d)
    nc.compile()
    _CACHED_NC = nc
    return nc


def _prep_host(x, gn_scale, gn_bias, qkv_w, qkv_b, proj_w, proj_b):
    """Host-side weight prep + per-core input maps."""
    f = np.float32
    x = np.asarray(x, f).reshape(B, C, N)
    qkv_w = np.asarray(qkv_w, f)
    qkv_b = np.asarray(qkv_b, f)
    proj_w = np.asarray(proj_w, f)
    proj_b = np.asarray(proj_b, f)
    scale = 1.0 / np.sqrt(np.float32(C))

    Wq, bq = qkv_w[0::3] * scale, qkv_b[0::3] * scale
    Wk, bk = qkv_w[1::3], qkv_b[1::3]
    Wv, bv = qkv_w[2::3], qkv_b[2::3]

    wqT = np.ascontiguousarray(Wq.T, f)
    wkT = np.ascontiguousarray(Wk.T, f)
    wvT = np.ascontiguousarray(Wv.T, f)
    pT = np.ascontiguousarray(proj_w.T, f)
    pbe = (proj_b + proj_w @ bv).astype(f)
    # vecs partition-major: vecs[p, v*2 + j] = vec_v[j*128 + p]
    vstack = np.stack([np.asarray(gn_scale, f), np.asarray(gn_bias, f),
                       bq.astype(f), bk.astype(f), pbe], axis=0)  # [5, 256]
    vecs = np.ascontiguousarray(
        vstack.reshape(5, 2, 128).transpose(2, 0, 1).reshape(128, 10))
    gind, gindT = _indicator_constants()

    shared = {"wqT": wqT, "wkT": wkT, "wvT": wvT, "pT": pT, "vecs": vecs,
              "gind": gind, "gindT": gindT}
    in_maps = []
    for ci in range(NCORES):
        b, half = divmod(ci, 2)
        xb = x[b]
        if half == 1:
            xb = np.concatenate([xb[:, NQ:], xb[:, :NQ]], axis=1)
        in_maps.append({"x": np.ascontiguousarray(xb), **shared})
    return in_maps


def _assemble(results):
    out = np.empty((B, C, N), np.float32)
    for ci in range(NCORES):
        b, half = divmod(ci, 2)
        out[b][:, half * NQ:(half + 1) * NQ] = results[ci]["out"]
    return out.reshape(B, C, H, W)


def kernel(x, gn_scale, gn_bias, qkv_w, qkv_b, proj_w, proj_b):
    nc = _build_program()
    in_maps = _prep_host(x, gn_scale, gn_bias, qkv_w, qkv_b, proj_w, proj_b)
    res = run_bass_kernel_spmd(nc, in_maps, core_ids=list(range(NCORES)))
    return _assemble(res.results)


if __name__ == "__main__":
    # smoke test with random data
    rng = np.random.default_rng(0)
    inputs = {
        "x": rng.standard_normal((B, C, H, W), dtype=np.float32),
        "gn_scale": np.ones(C, np.float32),
        "gn_bias": np.zeros(C, np.float32),
        "qkv_w": rng.standard_normal((3 * C, C), dtype=np.float32) * C ** -0.5,
        "qkv_b": np.zeros(3 * C, np.float32),
        "proj_w": rng.standard_normal((C, C), dtype=np.float32) * C ** -0.5,
        "proj_b": np.zeros(C, np.float32),
    }
    out = kernel(**inputs)
    print("out", out.shape, out.dtype, float(np.abs(out).mean()))



# revision 13
# speedup vs baseline: 1.9593x; 1.9593x over previous
"""Trainium2 Bass kernel for nn_AttnBlock (GroupNorm + single-head attention + proj + residual).

Reference computation (per batch element b, with C=256 channels, N=64*64=4096 positions):
    h   = GroupNorm32(x) * gn_scale + gn_bias
    q,k,v = split(qkv_w @ h + qkv_b)          (channel-interleaved split: rows 3c+0/1/2)
    w   = softmax_k(q^T k / sqrt(C))          [N, N]
    a   = v @ w^T                             [C, N]
    out = proj_w @ a + proj_b + x

Sharding: 8 cores = 4 batches x 2 q-halves.  Each core gets one full batch
element (needed for GroupNorm stats and full k/v), rolled so that its own
q-half occupies columns 0:2048; it computes the attention output for those
2048 query positions only.

fp8 DoubleRow design (all big matmuls in fp8e4 with perf_mode=DoubleRow,
contracting 2x128 per pass at 0.5 cyc/row):
  - Weights are quantized to fp8 on the HOST with power-of-2-ish scales:
    wq8 = fp8(alpha Wq), wk8 = fp8(alpha Wk) with alpha^2 = KAPPA/16 so the
    score psum comes out as KAPPA * s_true (KAPPA = 8*log2(e)); wv8 =
    fp8(4 Wv), pt8 = fp8(4 P).
  - GN is folded into the DATA: xdr = fp8(x * scale_c) (per-channel GN scale),
    so weights need no on-chip fold.  GN mean correction enters q via a bias
    (ACT bias on the q psum->fp8 copy); the k-side mean/bias correction
    cancels exactly in softmax (constant per query column, and we normalize
    with a rowsum computed from the same eT).  The v-side bias is folded into
    the output bias ob analytically.
  - exp: e^(s-2) computed per score pair-tile EITHER exactly on ACT
    (func=Exp, scale=1/KAPPA, bias=-2) with fp8 output, OR via a one-
    instruction Schraudolph trick on DVE/Pool: u8 = trunc(max(s + C_SCHR, 0))
    whose bits ARE the fp8e4 encoding of ~e^(s/KAPPA - 2).  The -2 shift
    keeps eT <= ~36 (fp8e4 max 240).  Engine choice per tile load-balances
    ACT/DVE/Pool.
  - Softmax normalization is deferred past the output projection (linear):
    po = pt8^T an is divided by the rowsum AFTER the proj matmul.  rowsum
    accumulates via a ones(=1/8) DoubleRow matmul into PSUM.
  - an = fp8(av/128): the 2^-7 scale keeps fp8 range; combined with the
    host weight scales, out = po/rs + ob + x needs no other factors.
"""

import numpy as np
import ml_dtypes

import concourse.bass as bass
import concourse.bacc as bacc
import concourse.tile as tile
from concourse import mybir
from concourse.bass_utils import run_bass_kernel_spmd

F32 = mybir.dt.float32
FP8 = mybir.dt.float8e4
U8 = mybir.dt.uint8
AF = mybir.ActivationFunctionType
OP = mybir.AluOpType
DR = mybir.MatmulPerfMode.DoubleRow

B, C, H, W = 4, 256, 64, 64
N = H * W               # 4096 positions
NQ = N // 2             # 2048 query positions per core
GROUPS = 32
GSIZE = C // GROUPS     # 8 channels per group
EPS = 1e-6
QB = 512                # query block (one PSUM bank of fp32)
NJB = NQ // QB          # 4 query blocks
KT = N // 128           # 32 k-position tiles
NPAIR = KT // 2         # 16 k-tile pairs per query block
NCORES = 8

LOG2E = 1.4426950408889634
KAPPA = 8.0 * LOG2E                  # score psum scale: s_psum = KAPPA*s_true
ALPHA = np.sqrt(KAPPA) / 4.0         # host q/k weight scale
C_SCHR = 56.0 - 16.0 * LOG2E + 0.5   # schraudolph offset (+0.5: trunc->round)

# exp engine schedule per k-tile pair (16 per query block).
# GPSIMD cannot read PSUM, so only ACT (exact exp) and DVE (schraudolph).
SCHED = ['act', 'dve', 'act', 'dve', 'act', 'dve', 'act', 'act',
         'dve', 'act', 'act', 'dve', 'act', 'act', 'dve', 'act']


def _indicator_constants():
    # gind: [128, 2, 32] (partition-major) with gind[p, t, g] = 1 iff
    #   group(t*128+p) == g;  gindT[t]: [32, 128] transpose (for broadcasting
    #   group stats back to channels)
    p = np.arange(128)
    gind = np.zeros((2, 128, 32), np.float32)
    for t in range(2):
        gind[t, p, t * 16 + p // GSIZE] = 1.0
    gindT = np.ascontiguousarray(np.transpose(gind, (0, 2, 1)))
    # gind pre-scaled by 1/GSIZE so the group-reduce matmul yields means
    gind_pmaj = np.ascontiguousarray(
        np.transpose(gind, (1, 0, 2))).reshape(128, 64) / GSIZE
    return gind_pmaj.astype(np.float32), gindT.reshape(2 * 32, 128)


def _emit(nc, tc, d):
    """Emit the per-core program. d: dict of DRAM APs."""
    x_d, wq_d, wk_d, wv_d, pt_d = d["x"], d["wq8"], d["wk8"], d["wv8"], d["pt8"]
    vec_d, out_d = d["vecs"], d["out"]
    gind_d, gindT_d = d["gind"], d["gindT"]

    import contextlib
    ctx = contextlib.ExitStack()
    with ctx:
        sing = ctx.enter_context(tc.tile_pool(name="sing", bufs=1))
        stat = ctx.enter_context(tc.tile_pool(name="stat", bufs=2))

        # ---- persistent SBUF tiles -------------------------------------
        x0 = sing.tile([128, N], F32, name="x0")
        x1 = sing.tile([128, N], F32, name="x1")
        xdr = sing.tile([128, 2, N], FP8, name="xdr")
        kdr = sing.tile([128, 2, N], FP8, name="kdr")
        qdr = sing.tile([128, 2, NQ], FP8, name="qdr")
        vt = sing.tile([128, KT, 256], FP8, name="vt")
        wq8 = sing.tile([128, 2, 256], FP8, name="wq8")
        wk8 = sing.tile([128, 2, 256], FP8, name="wk8")
        wv8 = sing.tile([128, 2, 256], FP8, name="wv8")
        pt8 = sing.tile([128, 2, 256], FP8, name="pt8")
        vecs = sing.tile([128, 4, 2], F32, name="vecs")  # gn_scale, gn_bias, qb, pbe
        gind = sing.tile([128, 2, 32], F32, name="gind")
        gindT0 = sing.tile([32, 128], F32, name="gindT0")
        gindT1 = sing.tile([32, 128], F32, name="gindT1")
        ones8 = sing.tile([128, 2, 16], FP8, name="ones8")
        gnb8 = sing.tile([128, 2, 16], FP8, name="gnb8")
        bv8 = sing.tile([128, 2, 16], FP8, name="bv8")
        epst = sing.tile([32, 1], F32, name="epst")
        nbias2 = sing.tile([128, 1], F32, name="nbias2")

        scale_c = sing.tile([128, 2], F32, name="scale_c")   # per-channel GN scale
        gnb_c = sing.tile([128, 2], F32, name="gnb_c")       # per-channel GN bias
        biasq = sing.tile([128, 2], F32, name="biasq")       # q bias per c_out
        ob_t = sing.tile([128, 2], F32, name="ob_t")         # final output bias

        # ---- DMAs -------------------------------------------------------
        XCH = 1024
        for c in range(N // XCH):
            csl = slice(c * XCH, (c + 1) * XCH)
            nc.sync.dma_start(out=x0[:, csl], in_=x_d[0:128, csl])
            nc.scalar.dma_start(out=x1[:, csl], in_=x_d[128:256, csl])
        for wt, wd in ((wq8, wq_d), (wk8, wk_d), (wv8, wv_d), (pt8, pt_d)):
            nc.gpsimd.dma_start(out=wt, in_=wd.rearrange("(j p) o -> p j o", p=128))
        nc.gpsimd.dma_start(out=vecs, in_=vec_d)
        nc.gpsimd.dma_start(out=gind, in_=gind_d)
        nc.gpsimd.dma_start(out=gindT0, in_=gindT_d[0:32, :])
        nc.gpsimd.dma_start(out=gindT1, in_=gindT_d[32:64, :])
        nc.vector.memset(ones8, 0.125)
        nc.vector.memset(epst, EPS)
        nc.vector.memset(nbias2, -2.0)

        gsc = vecs[:, 0, :]
        gbi = vecs[:, 1, :]
        qbv = vecs[:, 2, :]
        pbe = vecs[:, 3, :]

        # ---- phase 1: GroupNorm statistics ------------------------------
        with tc.tile_pool(name="ps_small", bufs=2, space="PSUM") as ps_small:
            bstats0 = stat.tile([128, GSIZE, 6], F32, name="bstats0", tag="bstats0", bufs=1)
            bstats1 = stat.tile([128, GSIZE, 6], F32, name="bstats1", tag="bstats1", bufs=1)
            for sg in range(GSIZE):
                nc.vector.bn_stats(out=bstats0[:, sg, :], in_=x0[:, sg * 512:(sg + 1) * 512])
                nc.vector.bn_stats(out=bstats1[:, sg, :], in_=x1[:, sg * 512:(sg + 1) * 512])
            statsin = []
            for t, bstats in enumerate((bstats0, bstats1)):
                mv = stat.tile([128, 2], F32, name=f"mv{t}", tag="mv")
                nc.vector.bn_aggr(out=mv, in_=bstats)
                si = stat.tile([128, 2], F32, name=f"si{t}", tag=f"si{t}", bufs=1)
                nc.vector.tensor_copy(out=si[:, 0:1], in_=mv[:, 0:1])
                nc.vector.tensor_tensor(out=si[:, 1:2], in0=mv[:, 0:1], in1=mv[:, 0:1], op=OP.mult)
                nc.vector.tensor_tensor(out=si[:, 1:2], in0=si[:, 1:2], in1=mv[:, 1:2], op=OP.add)
                statsin.append(si)

            gsum_ps = ps_small.tile([32, 2], F32, name="gsum_ps", tag="gsum")
            nc.tensor.matmul(gsum_ps, gind[:, 0, :], statsin[0], start=True, stop=False)
            nc.tensor.matmul(gsum_ps, gind[:, 1, :], statsin[1], start=False, stop=True)

            grp = stat.tile([32, 2], F32, name="grp", bufs=1)
            nc.vector.tensor_copy(out=grp, in_=gsum_ps)
            var_g = stat.tile([32, 1], F32, name="var_g", bufs=1)
            # mu^2 - E2 = -var, then sqrt(-1 * in + eps) = sqrt(var + eps)
            nc.vector.scalar_tensor_tensor(out=var_g, in0=grp[:, 0:1],
                                           scalar=grp[:, 0:1], in1=grp[:, 1:2],
                                           op0=OP.mult, op1=OP.subtract)
            nc.scalar.activation(out=var_g, in_=var_g, func=AF.Sqrt, bias=epst, scale=-1.0)
            nc.vector.reciprocal(out=grp[:, 1:2], in_=var_g)  # grp = (mu_g, rstd_g)

            for t, gt in enumerate((gindT0, gindT1)):
                bc_ps = ps_small.tile([128, 2], F32, name=f"bc_ps{t}", tag="bc")
                nc.tensor.matmul(bc_ps, gt, grp, start=True, stop=True)
                # scale_c = gn_scale * rstd ; gnb_c = gn_bias - mu * scale_c
                nc.vector.tensor_tensor(out=scale_c[:, t:t + 1], in0=gsc[:, t:t + 1],
                                        in1=bc_ps[:, 1:2], op=OP.mult)
                nc.vector.tensor_tensor(out=gnb_c[:, t:t + 1], in0=bc_ps[:, 0:1],
                                        in1=scale_c[:, t:t + 1], op=OP.mult)
                nc.vector.tensor_tensor(out=gnb_c[:, t:t + 1], in0=gbi[:, t:t + 1],
                                        in1=gnb_c[:, t:t + 1], op=OP.subtract)
            nc.vector.tensor_copy(out=gnb8[:, :, 0], in_=gnb_c)

            # ---- phase 2: xdr = fp8(x * scale_c), 512-wide slices over
            # ACT/DVE so k-proj blocks can start as slices finish (Pool
            # supports neither PSUM reads nor AP-scalar TensorScalar)
            for sl8 in range(8):
                psl = slice(sl8 * 512, (sl8 + 1) * 512)
                for j, xsrc in enumerate((x0, x1)):
                    if (sl8 * 2 + j) % 2 == 0:
                        nc.scalar.activation(out=xdr[:, j, psl], in_=xsrc[:, psl],
                                             func=AF.Copy, bias=0.0,
                                             scale=scale_c[:, j:j + 1])
                    else:
                        nc.vector.tensor_scalar_mul(out=xdr[:, j, psl], in0=xsrc[:, psl],
                                                    scalar1=scale_c[:, j:j + 1])

            # ---- bias chains (tiny DoubleRow matmuls with fp8 weights) ---
            # biasq = wq8^T gnb8 + alpha*qkv_bq ; bv8 = fp8(wv8^T gnb8)
            # ob = pbe + (pt8^T bv8)/16
            for ot in range(2):
                osl = slice(ot * 128, (ot + 1) * 128)
                bq_ps = ps_small.tile([128, 1], F32, name=f"bq_ps{ot}", tag="b_ps")
                nc.tensor.matmul(bq_ps, wq8[:, :, osl], gnb8[:, :, 0:1],
                                 start=True, stop=True, perf_mode=DR)
                nc.vector.tensor_tensor(out=biasq[:, ot:ot + 1], in0=bq_ps,
                                        in1=qbv[:, ot:ot + 1], op=OP.add)
                bv_ps = ps_small.tile([128, 1], F32, name=f"bv_ps{ot}", tag="b_ps")
                nc.tensor.matmul(bv_ps, wv8[:, :, osl], gnb8[:, :, 0:1],
                                 start=True, stop=True, perf_mode=DR)
                nc.vector.tensor_copy(out=bv8[:, ot, 0:1], in_=bv_ps)
            for ot in range(2):
                osl = slice(ot * 128, (ot + 1) * 128)
                d_ps = ps_small.tile([128, 1], F32, name=f"d_ps{ot}", tag="b_ps")
                nc.tensor.matmul(d_ps, pt8[:, :, osl], bv8[:, :, 0:1],
                                 start=True, stop=True, perf_mode=DR)
                nc.scalar.activation(out=ob_t[:, ot:ot + 1], in_=d_ps, func=AF.Identity,
                                     bias=pbe[:, ot:ot + 1], scale=0.0625)

        # ---- phase 3: k / q / v projections (all fp8 DoubleRow) ---------
        with (
            tc.tile_pool(name="ps_kq", bufs=3, space="PSUM") as ps_kq,
            tc.tile_pool(name="ps_v", bufs=2, space="PSUM") as ps_v,
        ):
            # k: 8 blocks of 512 positions; psum pair (ot0/ot1) -> one copy
            for kb in range(8):
                psl = slice(kb * 512, (kb + 1) * 512)
                kp = ps_kq.tile([128, 2, 512], F32, name="kp", tag="kq")
                nc.tensor.matmul(kp[:, 0, :], wk8[:, :, 0:128], xdr[:, :, psl],
                                 start=True, stop=True, perf_mode=DR)
                nc.tensor.matmul(kp[:, 1, :], wk8[:, :, 128:256], xdr[:, :, psl],
                                 start=True, stop=True, perf_mode=DR)
                if kb % 2 == 0:
                    nc.vector.tensor_copy(out=kdr[:, :, psl], in_=kp)
                else:
                    nc.scalar.copy(out=kdr[:, :, psl], in_=kp)
            # q: 4 blocks; bias via ACT (per-ot halves)
            for jb in range(NJB):
                qsl = slice(jb * QB, (jb + 1) * QB)
                qp = ps_kq.tile([128, 2, 512], F32, name="qp", tag="kq")
                for ot in range(2):
                    osl = slice(ot * 128, (ot + 1) * 128)
                    nc.tensor.matmul(qp[:, ot, :], wq8[:, :, osl], xdr[:, :, qsl],
                                     start=True, stop=True, perf_mode=DR)
                    nc.scalar.activation(out=qdr[:, ot, qsl], in_=qp[:, ot, :],
                                         func=AF.Identity, bias=biasq[:, ot:ot + 1],
                                         scale=1.0)
            # v: 16 pairs of 128-position tiles; one bank per pair
            for vp in range(NPAIR):
                vps = ps_v.tile([128, 2, 256], F32, name="vps", tag="v")
                for i in range(2):
                    nsl = slice((2 * vp + i) * 128, (2 * vp + i + 1) * 128)
                    nc.tensor.matmul(vps[:, i, :], xdr[:, :, nsl], wv8,
                                     start=True, stop=True, perf_mode=DR)
                if vp % 2 == 0:
                    nc.vector.tensor_copy(out=vt[:, 2 * vp:2 * vp + 2, :], in_=vps)
                else:
                    nc.scalar.copy(out=vt[:, 2 * vp:2 * vp + 2, :], in_=vps)

        # ---- phase 4: attention -----------------------------------------
        with (
            tc.tile_pool(name="ps_s", bufs=2, space="PSUM") as ps_s,
            tc.tile_pool(name="ps_av", bufs=2, space="PSUM") as ps_av,
            tc.tile_pool(name="ps_rs", bufs=1, space="PSUM") as ps_rs,
            tc.tile_pool(name="ps_po", bufs=1, space="PSUM") as ps_po,
            tc.tile_pool(name="eT_pool", bufs=4) as eT_pool,
            tc.tile_pool(name="an_pool", bufs=2) as an_pool,
            tc.tile_pool(name="o_pool", bufs=4) as o_pool,
            tc.tile_pool(name="rs_pool", bufs=2) as rs_pool,
        ):
            def epilogue(jb, andr, rsb):
                # proj + normalize + bias/residual + store for query block jb
                qsl = slice(jb * QB, (jb + 1) * QB)
                for ot, xres in enumerate((x0, x1)):
                    osl = slice(ot * 128, (ot + 1) * 128)
                    po = ps_po.tile([128, QB], F32, name="po", tag="po")
                    nc.tensor.matmul(po, pt8[:, :, osl], andr,
                                     start=True, stop=True, perf_mode=DR)
                    t1 = o_pool.tile([128, QB], F32, name="t1", tag="t1")
                    nc.vector.tensor_tensor(out=t1, in0=po, in1=rsb, op=OP.mult)
                    o_sb = o_pool.tile([128, QB], F32, name="o_sb", tag="o_sb")
                    nc.vector.scalar_tensor_tensor(out=o_sb, in0=t1,
                                                   scalar=ob_t[:, ot:ot + 1],
                                                   in1=xres[:, qsl],
                                                   op0=OP.add, op1=OP.add)
                    nc.sync.dma_start(out=out_d[osl, qsl], in_=o_sb)

            pending = None
            for jb in range(NJB):
                qsl = slice(jb * QB, (jb + 1) * QB)
                av_a = ps_av.tile([128, QB], F32, name="av_a", tag="av")
                av_b = ps_av.tile([128, QB], F32, name="av_b", tag="av")
                rs = ps_rs.tile([128, QB], F32, name="rs", tag="rs")
                eTs = {}

                def av_group(t, av_a=av_a, av_b=av_b, rs=rs, eTs=eTs):
                    eT8 = eTs.pop(t).bitcast(FP8)
                    st, sp = (t == 0), (t == NPAIR - 1)
                    nc.tensor.matmul(rs[0:1, :], ones8[:, :, 0:1], eT8,
                                     start=st, stop=sp, perf_mode=DR)
                    nc.tensor.matmul(av_a, vt[:, 2 * t:2 * t + 2, 0:128], eT8,
                                     start=st, stop=sp, perf_mode=DR)
                    nc.tensor.matmul(av_b, vt[:, 2 * t:2 * t + 2, 128:256], eT8,
                                     start=st, stop=sp, perf_mode=DR)

                for t in range(NPAIR):
                    s_pair = ps_s.tile([128, 2, QB], F32, name="s_pair", tag="s")
                    for i in range(2):
                        ksl = slice((2 * t + i) * 128, (2 * t + i + 1) * 128)
                        nc.tensor.matmul(s_pair[:, i, :], kdr[:, :, ksl],
                                         qdr[:, :, qsl], start=True, stop=True,
                                         perf_mode=DR)
                    eT = eT_pool.tile([128, 2, QB], U8, name="eT", tag="eT")
                    eng = SCHED[t]
                    if eng == 'act':
                        nc.scalar.activation(out=eT.bitcast(FP8), in_=s_pair,
                                             func=AF.Exp, bias=nbias2,
                                             scale=1.0 / KAPPA)
                    else:
                        nc.vector.tensor_scalar(out=eT, in0=s_pair, scalar1=C_SCHR,
                                                scalar2=0.0, op0=OP.add, op1=OP.max)
                    eTs[t] = eT
                    if t >= 2:
                        av_group(t - 2)
                    if t == 4 and pending is not None:
                        epilogue(*pending)
                        pending = None
                av_group(NPAIR - 2)
                av_group(NPAIR - 1)

                if jb < NJB - 1:
                    # handoff: an = fp8(av/128); rsb = bcast(1/rs)
                    andr = an_pool.tile([128, 2, QB], FP8, name="andr", tag="an")
                    nc.scalar.activation(out=andr[:, 0, :], in_=av_a, func=AF.Identity,
                                         bias=0.0, scale=1.0 / 128.0)
                    nc.scalar.activation(out=andr[:, 1, :], in_=av_b, func=AF.Identity,
                                         bias=0.0, scale=1.0 / 128.0)
                    rsr = rs_pool.tile([1, QB], F32, name="rsr", tag="rsr")
                    nc.vector.reciprocal_approx_fast(out=rsr, in_=rs[0:1, :])
                    rsb = rs_pool.tile([128, QB], F32, name="rsb", tag="rsb")
                    nc.gpsimd.partition_broadcast(rsb, rsr)
                    pending = (jb, andr, rsb)
                else:
                    # final block: no following PE work to hide behind ->
                    # pipeline the tail in two half-width pieces
                    HB = QB // 2
                    for h in range(2):
                        hsl = slice(h * HB, (h + 1) * HB)
                        qsl_h = slice(jb * QB + h * HB, jb * QB + (h + 1) * HB)
                        an_h = an_pool.tile([128, 2, HB], FP8, name=f"an_h{h}", tag="an")
                        nc.scalar.activation(out=an_h[:, 0, :], in_=av_a[:, hsl],
                                             func=AF.Identity, bias=0.0, scale=1.0 / 128.0)
                        nc.scalar.activation(out=an_h[:, 1, :], in_=av_b[:, hsl],
                                             func=AF.Identity, bias=0.0, scale=1.0 / 128.0)
                        rsr_h = rs_pool.tile([1, HB], F32, name=f"rsrh{h}", tag=f"rsrh{h}", bufs=1)
                        nc.vector.reciprocal_approx_fast(out=rsr_h, in_=rs[0:1, hsl])
                        rsb_h = rs_pool.tile([128, HB], F32, name=f"rsbh{h}", tag=f"rsbh{h}", bufs=1)
                        nc.gpsimd.partition_broadcast(rsb_h, rsr_h)
                        for ot, xres in enumerate((x0, x1)):
                            osl = slice(ot * 128, (ot + 1) * 128)
                            po = ps_po.tile([128, HB], F32, name="po_h", tag="po")
                            nc.tensor.matmul(po, pt8[:, :, osl], an_h,
                                             start=True, stop=True, perf_mode=DR)
                            t1 = o_pool.tile([128, HB], F32, name="t1_h", tag="t1")
                            nc.vector.tensor_tensor(out=t1, in0=po, in1=rsb_h, op=OP.mult)
                            o_sb = o_pool.tile([128, HB], F32, name="o_sb_h", tag="o_sb")
                            nc.vector.scalar_tensor_tensor(out=o_sb, in0=t1,
                                                           scalar=ob_t[:, ot:ot + 1],
                                                           in1=xres[:, qsl_h],
                                                           op0=OP.add, op1=OP.add)
                            nc.sync.dma_start(out=out_d[osl, qsl_h], in_=o_sb)
            assert pending is None


_CACHED_NC = None


def _build_program():
    global _CACHED_NC
    if _CACHED_NC is not None:
        return _CACHED_NC
    nc = bacc.Bacc("TRN2", target_bir_lowering=False, debug=False,
                   num_devices=NCORES)
    d = {
        "x": nc.dram_tensor("x", [C, N], F32, kind="ExternalInput").ap(),
        "wq8": nc.dram_tensor("wq8", [C, C], FP8, kind="ExternalInput").ap(),
        "wk8": nc.dram_tensor("wk8", [C, C], FP8, kind="ExternalInput").ap(),
        "wv8": nc.dram_tensor("wv8", [C, C], FP8, kind="ExternalInput").ap(),
        "pt8": nc.dram_tensor("pt8", [C, C], FP8, kind="ExternalInput").ap(),
        "vecs": nc.dram_tensor("vecs", [128, 8], F32, kind="ExternalInput").ap(),
        "gind": nc.dram_tensor("gind", [128, 64], F32, kind="ExternalInput").ap(),
        "gindT": nc.dram_tensor("gindT", [2 * 32, 128], F32, kind="ExternalInput").ap(),
        "out": nc.dram_tensor("out", [C, NQ], F32, kind="ExternalOutput").ap(),
    }
    with tile.TileContext(nc) as tc:
        _emit(nc, tc, d)
    nc.compile()
    _CACHED_NC = nc
    return nc


def _prep_host(x, gn_scale, gn_bias, qkv_w, qkv_b, proj_w, proj_b):
    """Host-side weight prep + per-core input maps."""
    f = np.float32
    f8 = ml_dtypes.float8_e4m3
    x = np.asarray(x, f).reshape(B, C, N)
    qkv_w = np.asarray(qkv_w, f)
    qkv_b = np.asarray(qkv_b, f)
    proj_w = np.asarray(proj_w, f)
    proj_b = np.asarray(proj_b, f)

    Wq, bq = qkv_w[0::3], qkv_b[0::3]
    Wk = qkv_w[1::3]
    Wv, bv = qkv_w[2::3], qkv_b[2::3]

    a = np.float32(ALPHA)
    wq8 = np.ascontiguousarray((a * Wq).T).astype(f8)
    wk8 = np.ascontiguousarray((a * Wk).T).astype(f8)
    wv8 = np.ascontiguousarray((4.0 * Wv).T.astype(f)).astype(f8)
    pt8 = np.ascontiguousarray((4.0 * proj_w).T.astype(f)).astype(f8)
    pbe = (proj_b + proj_w @ bv).astype(f)
    qb = (a * bq).astype(f)
    # vecs partition-major: vecs[p, v*2 + j] = vec_v[j*128 + p]
    vstack = np.stack([np.asarray(gn_scale, f), np.asarray(gn_bias, f),
                       qb, pbe], axis=0)  # [4, 256]
    vecs = np.ascontiguousarray(
        vstack.reshape(4, 2, 128).transpose(2, 0, 1).reshape(128, 8))
    gind, gindT = _indicator_constants()

    shared = {"wq8": wq8, "wk8": wk8, "wv8": wv8, "pt8": pt8, "vecs": vecs,
              "gind": gind, "gindT": gindT}
    in_maps = []
    for ci in range(NCORES):
        b, half = divmod(ci, 2)
        xb = x[b]
        if half == 1:
            xb = np.concatenate([xb[:, NQ:], xb[:, :NQ]], axis=1)
        in_maps.append({"x": np.ascontiguousarray(xb), **shared})
    return in_maps


def _assemble(results):
    out = np.empty((B, C, N), np.float32)
    for ci in range(NCORES):
        b, half = divmod(ci, 2)
        out[b][:, half * NQ:(half + 1) * NQ] = results[ci]["out"]
    return out.reshape(B, C, H, W)


def kernel(x, gn_scale, gn_bias, qkv_w, qkv_b, proj_w, proj_b):
    nc = _build_program()
    in_maps = _prep_host(x, gn_scale, gn_bias, qkv_w, qkv_b, proj_w, proj_b)
    res = run_bass_kernel_spmd(nc, in_maps, core_ids=list(range(NCORES)))
    return _assemble(res.results)


if __name__ == "__main__":
    # smoke test with random data
    rng = np.random.default_rng(0)
    inputs = {
        "x": rng.standard_normal((B, C, H, W), dtype=np.float32),
        "gn_scale": np.ones(C, np.float32),
        "gn_bias": np.zeros(C, np.float32),
        "qkv_w": rng.standard_normal((3 * C, C), dtype=np.float32) * C ** -0.5,
        "qkv_b": np.zeros(3 * C, np.float32),
        "proj_w": rng.standard_normal((C, C), dtype=np.float32) * C ** -0.5,
        "proj_b": np.zeros(C, np.float32),
    }
    out = kernel(**inputs)
    print("out", out.shape, out.dtype, float(np.abs(out).mean()))
